# revision 14
# baseline (speedup 1.0000x reference)
"""LIF recurrent spiking network on 8 TRN2 NeuronCores.

Strategy: tensor-parallel shard of W_rec columns (256 per core). Each step:
  psum = x_t @ W_in[:, shard] + z @ (W_rec - I)[:, shard]      (TensorE)
  v    = DECAY*v + psum + noise_t[:, shard]                    (DVE)
  z    = (v > 1.0)                                             (DVE, bf16)
  all-gather z^T (bf16) across the 8 cores for the next step.

Weights are split into bf16 hi+lo pairs, concatenated into one 512-wide
moving operand per k-chunk: z and x are exactly {0,1} in bf16, so products
are exact and the only error is fp32 PSUM accumulation order — the same
chaos floor any reassociated fp32 implementation hits.

Self-contained: builds the Bass program, shards inputs, runs via
bass_utils.run_bass_kernel_spmd on cores 0-7, reassembles full outputs.
"""

import numpy as np
import ml_dtypes

import concourse.bass as bass
import concourse.mybir as mybir
from concourse import bass_utils
from concourse.tile import TileContext
from concourse.masks import make_identity

# ---------------------------------------------------------------- constants
B = 64          # batch
T_FULL = 500    # time steps
NIN = 256       # input neurons
N = 2048        # recurrent neurons
NCORES = 8
NJ = N // NCORES          # output-column shard per core (256)
KC = N // 128             # k-chunks over recurrent contraction (16)
KI = NIN // 128           # k-chunks over input contraction (2)
KL = KC // NCORES         # k-chunks produced per core (2)
NG = 2                    # batch groups (software pipelining over the gather)
BG = B // NG              # batch per group (32)
DECAY = float(np.exp(-1.0 / 20.0))
THR = 1.0

F32 = mybir.dt.float32
BF16 = mybir.dt.bfloat16
AG_DT = BF16              # dtype for the z^T all-gather

_NC_CACHE = {}


# ---------------------------------------------------------------- tile patch
def _patch_tile_drain():
    """This walrus build accepts only ONE sync-wait on the CTRL/NO_STRUCT
    template; TileContext's kernel-tail Drain collects many. Split them."""
    if getattr(TileContext, "_drain_patched", False):
        return

    def _drain_and_barrier(self, tick_clock, wait_clock):
        from concourse.tile import ScopedClock

        drain_inst = self.nc.sync.drain()
        wait_clock.add_sem_waits(
            drain_inst.ins, ScopedClock({None: tick_clock.global_clock})
        )
        si = drain_inst.ins.sync_info
        waits = list(si.on_wait) if si is not None else []
        if len(waits) > 1:
            si.on_wait = waits[:1]
            for w in waits[1:]:
                extra = self.nc.sync.drain()
                esi = extra.ins.sync_info
                if esi is None:
                    esi = mybir.SyncInfo(on_wait=[], on_update=[])
                    extra.ins.sync_info = esi
                esi.on_wait = [w]
        self.nc.all_engine_barrier()
        assert self.sems is not None
        popped = self.nc._tile_sem_poison_stack.pop()
        assert popped is self._sem_poison
        self.nc.clear_and_free_semaphores(list(self.sems.allocated().values()))
        self.nc.all_engine_barrier()

    TileContext._drain_and_barrier = _drain_and_barrier
    TileContext._drain_patched = True


def _split_waits_json(raw: bytes) -> bytes:
    """This walrus build rejects instructions carrying more than one sync
    wait ("Too many sync wait commands"). Hoist extra waits onto injected
    EventSemaphore instructions placed immediately before, same engine."""
    import json

    m = json.loads(raw)
    uid = [0]
    for f in m.get("functions", []):
        for blk in f.get("blocks", []):
            insts = blk.get("instructions", [])
            out = []
            for inst in insts:
                si = inst.get("sync_info")
                waits = (si or {}).get("on_wait") or []
                if len(waits) > 1:
                    for w in waits[:-1]:
                        uid[0] += 1
                        out.append({
                            "engine": inst["engine"],
                            "ins": [],
                            "outs": [],
                            "name": f"wsplit-{uid[0]}",
                            "opcode": "EventSemaphore",
                            "debug": inst.get("debug", 0),
                            "sync_info": {"on_update": [], "on_wait": [w]},
                        })
                    si["on_wait"] = waits[-1:]
                out.append(inst)
            blk["instructions"] = out
    return json.dumps(m).encode()


def _wrap_json(nc):
    orig = nc.to_json_bytes

    def to_json_bytes(*a, **kw):
        return _split_waits_json(orig(*a, **kw))

    nc.to_json_bytes = to_json_bytes
    return nc


# ---------------------------------------------------------------- bass build
def build_nc(T):
    _patch_tile_drain()
    nc = bass.Bass(target_bir_lowering=False)

    # weights: hi|lo pair concatenated on the free dim -> [*, 128, 2*NJ]
    wr = nc.declare_dram_parameter("wr", [KC, 128, 2 * NJ], BF16, isOutput=False)
    wi = nc.declare_dram_parameter("wi", [KI, 128, 2 * NJ], BF16, isOutput=False)
    xT = [nc.declare_dram_parameter(f"xT{g}", [T, KI, 128, BG], BF16,
                                    isOutput=False) for g in range(NG)]
    noise_d = [nc.declare_dram_parameter(f"noise{g}", [T, BG, NJ], F32,
                                         isOutput=False) for g in range(NG)]
    v_out = [nc.declare_dram_parameter(f"v_out{g}", [T, BG, NJ], F32,
                                       isOutput=True) for g in range(NG)]
    z_out = [nc.declare_dram_parameter(f"z_out{g}", [T, BG, NJ], BF16,
                                       isOutput=True) for g in range(NG)]

    groups = [list(range(NCORES))]
    ADD = mybir.AluOpType.add

    with TileContext(nc) as tc:
        with (
            tc.tile_pool(name="persist", bufs=1) as persist,
            tc.tile_pool(name="xt", bufs=4) as xt_pool,
            tc.tile_pool(name="noise", bufs=4) as noise_pool,
            tc.tile_pool(name="zbf", bufs=4) as zbf_pool,
            tc.tile_pool(name="ccin_sb", bufs=4) as ccin_sb_pool,
            tc.tile_pool(name="ccd", bufs=4, space="DRAM") as cc_pool,
            tc.tile_pool(name="psum", bufs=2, space="PSUM") as psum_pool,
            tc.tile_pool(name="tpsum", bufs=2, space="PSUM") as tpsum_pool,
        ):
            wr_sb = persist.tile([128, KC, 2 * NJ], BF16, name="wr_sb")
            wi_sb = persist.tile([128, KI, 2 * NJ], BF16, name="wi_sb")
            zT_sb = [persist.tile([128, KC, BG], AG_DT, name=f"zT_sb{g}")
                     for g in range(NG)]
            v_sb = [persist.tile([BG, NJ], F32, name=f"v_sb{g}")
                    for g in range(NG)]
            ident = persist.tile([128, 128], BF16, name="ident")

            # one-time setup (per-chunk DMAs)
            for kc in range(KC):
                nc.sync.dma_start(out=wr_sb[:, kc, :], in_=wr[kc])
            for ki in range(KI):
                nc.sync.dma_start(out=wi_sb[:, ki, :], in_=wi[ki])
            make_identity(nc, ident[:])
            for g in range(NG):
                nc.vector.memset(v_sb[g][:], 0.0)

            for t in range(T):
                for g in range(NG):
                    xt = xt_pool.tile([128, KI, BG], BF16, tag=f"xt{g}")
                    nc.scalar.dma_start(
                        out=xt[:], in_=xT[g][t].rearrange("k p b -> p k b"))
                    noise_t = noise_pool.tile([BG, NJ], F32, tag=f"noise{g}")
                    nc.scalar.dma_start(out=noise_t[:], in_=noise_d[g][t])

                    psum = psum_pool.tile([BG, 2 * NJ], F32, tag=f"psum{g}")
                    # input projection first: no dependency on z -> overlaps
                    # the previous gather
                    mms = [(xt[:, ki, :], wi_sb[:, ki, :]) for ki in range(KI)]
                    if t > 0:
                        mms += [(zT_sb[g][:, kc, :], wr_sb[:, kc, :])
                                for kc in range(KC)]
                    for i, (lhsT, rhs) in enumerate(mms):
                        nc.tensor.matmul(
                            psum[:], lhsT, rhs,
                            start=(i == 0), stop=(i == len(mms) - 1),
                        )

                    # v = DECAY*v + psum_hi + psum_lo + noise
                    nc.vector.scalar_tensor_tensor(
                        out=v_sb[g][:], in0=v_sb[g][:], scalar=DECAY,
                        in1=psum[:, :NJ],
                        op0=mybir.AluOpType.mult, op1=ADD,
                    )
                    nc.vector.tensor_add(v_sb[g][:], v_sb[g][:], psum[:, NJ:])
                    nc.vector.tensor_add(v_sb[g][:], v_sb[g][:], noise_t[:])
                    nc.gpsimd.dma_start(out=v_out[g][t], in_=v_sb[g][:])

                    # z = (v > THR) directly in bf16 (exact 0/1)
                    zbf = zbf_pool.tile([BG, NJ], AG_DT, tag=f"zbf{g}")
                    nc.vector.tensor_scalar(
                        out=zbf[:], in0=v_sb[g][:], scalar1=THR, scalar2=None,
                        op0=mybir.AluOpType.is_gt,
                    )
                    nc.gpsimd.dma_start(out=z_out[g][t], in_=zbf[:])

                    if t == T - 1:
                        continue  # last step: no gather needed

                    # transpose own shard [BG, 2*128] -> [2, 128, BG], gather
                    ccin_sb = ccin_sb_pool.tile([128, KL, BG], AG_DT,
                                                tag=f"ccin{g}")
                    for i in range(KL):
                        pt = tpsum_pool.tile([128, BG], AG_DT, tag=f"pt{g}")
                        nc.tensor.transpose(
                            pt[:], zbf[:, i * 128:(i + 1) * 128],
                            ident[:BG, :BG]
                        )
                        nc.vector.tensor_copy(ccin_sb[:, i, :], pt[:])
                    ccin_d = cc_pool.tile([128, KL, BG], AG_DT, tag=f"ccin{g}")
                    ccout_d = cc_pool.tile(
                        [NCORES, 128, KL, BG], AG_DT, tag=f"ccout{g}",
                        addr_space="Shared"
                    )
                    nc.sync.dma_start(out=ccin_d[:], in_=ccin_sb[:])
                    nc.gpsimd.collective_compute(
                        "AllGather",
                        mybir.AluOpType.bypass,
                        replica_groups=groups,
                        ins=[ccin_d[:]],
                        outs=[ccout_d[:]],
                    )
                    # rank-major gather -> [128, (rank, kl), BG] = [128, KC, BG]
                    nc.sync.dma_start(
                        out=zT_sb[g][:],
                        in_=ccout_d[:].rearrange("r p k b -> p r k b")
                    )
    return _wrap_json(nc)


# ---------------------------------------------------------------- host side
def _pair(w):
    hi = w.astype(ml_dtypes.bfloat16)
    lo = (w - hi.astype(np.float32)).astype(ml_dtypes.bfloat16)
    return hi, lo


def prepare_inputs(x, noise, W_in, W_rec, T):
    """Returns in_maps (list of 8 dicts)."""
    x = np.asarray(x, np.float32)[:, :T]
    noise = np.asarray(noise, np.float32)[:, :T]
    W_in = np.asarray(W_in, np.float32)
    W_rec = np.asarray(W_rec, np.float32)

    Wp = W_rec - THR * np.eye(N, dtype=np.float32)  # fold soft reset into W
    wr_hi, wr_lo = _pair(Wp)
    wi_hi, wi_lo = _pair(W_in)

    in_maps = []
    for c in range(NCORES):
        sl = slice(c * NJ, (c + 1) * NJ)
        wr_c = np.concatenate(
            [wr_hi[:, sl].reshape(KC, 128, NJ), wr_lo[:, sl].reshape(KC, 128, NJ)],
            axis=2,
        )
        wi_c = np.concatenate(
            [wi_hi[:, sl].reshape(KI, 128, NJ), wi_lo[:, sl].reshape(KI, 128, NJ)],
            axis=2,
        )
        im = {"wr": np.ascontiguousarray(wr_c), "wi": np.ascontiguousarray(wi_c)}
        for g in range(NG):
            bs = slice(g * BG, (g + 1) * BG)
            im[f"xT{g}"] = np.ascontiguousarray(
                x[bs].transpose(1, 2, 0).reshape(T, KI, 128, BG)
            ).astype(ml_dtypes.bfloat16)
            im[f"noise{g}"] = np.ascontiguousarray(
                noise[bs, :, sl].transpose(1, 0, 2))
        in_maps.append(im)
    return in_maps


def run(x, noise, W_in, W_rec, T=T_FULL, trace=False):
    key = T
    if key not in _NC_CACHE:
        _NC_CACHE[key] = build_nc(T)
    nc = _NC_CACHE[key]
    in_maps = prepare_inputs(x, noise, W_in, W_rec, T)
    res = bass_utils.run_bass_kernel_spmd(
        nc, in_maps, core_ids=list(range(NCORES)), trace=trace
    )
    voltages = np.empty((B, T, N), np.float32)
    spikes = np.empty((B, T, N), np.float32)
    for c in range(NCORES):
        sl = slice(c * NJ, (c + 1) * NJ)
        for g in range(NG):
            bs = slice(g * BG, (g + 1) * BG)
            voltages[bs, :, sl] = res.results[c][f"v_out{g}"].transpose(1, 0, 2)
            spikes[bs, :, sl] = (
                res.results[c][f"z_out{g}"].astype(np.float32).transpose(1, 0, 2))
    return (voltages, spikes), res


def kernel(x, noise, W_in, W_rec):
    (voltages, spikes), _ = run(x, noise, W_in, W_rec, T=T_FULL, trace=False)
    return voltages, spikes
